# revision 18
# baseline (speedup 1.0000x reference)
"""ARX forward kernel for Trainium2 (8 NeuronCores, data-parallel).

The reference zeroes the exogenous term, so the model is a pure linear
recurrence out[:, t] = sum_k w_k * out[:, t-8+k] with out[:, :8] = y.
Writing the 8x8 companion matrix M (carry_{t+1} = carry_t @ M) gives
pred_t = y @ (M^t w), so the whole 4096-step scan collapses into one
matmul out = y @ [I_8 | V] with V[:, t] = M^t w precomputed on host
(4096 tiny 8-vector iterations, float64).

The recurrence is stable (spectral radius ~0.77 for the 0.05-scaled
weights), so M^t w underflows float32 to exactly 0 after a few hundred
steps; both the reference scan and this kernel produce exact zeros
there.  The device therefore computes and writes only the nonzero
column prefix (determined from V at runtime) and the host pads the
remaining all-zero columns.

Sharding: pure data parallel, batch 8192 -> 1024 rows per core, W/V
replicated, per-core output gathered on host by concatenation.

Device kernel (raw bass, per core): the contraction dim is only 8, so
matmuls are packed 4x via TensorE row tiling (32x128 mode): row groups
at partitions 0/32/64/96 each hold one 128-row batch chunk's yT and a
replica of V, and 4 matmuls run concurrently in the array.  2 rounds
cover the 8 chunks; 8 PSUM banks hold the results, which DVE/ACT
copy to SBUF and HWDGE DMAs stream to DRAM.
"""

import os

import numpy as np

AR = 8
SEQ = 4096
BATCH = 8192
OUT_COLS = SEQ + AR          # 4104
N_CORES = 8
ROWS = BATCH // N_CORES      # 1024
P = 128                      # SBUF/PSUM partitions
MM_CHUNK = 512               # max fp32 matmul free dim / one PSUM bank
N_CHUNKS = ROWS // P         # 8 row chunks per core
N_GRP = 4                    # TensorE row groups (32-row tiling)
N_ROUNDS = N_CHUNKS // N_GRP

_nc_cache = {}
LAST_RESULTS = None          # BassKernelResults of the most recent run


def _build_nc_raw(n_cols, c32=None):
    """Raw-bass program: out[1024, n_cols] = y_shard @ [I|V] (per core).

    Input layout (host-packed, see _pack_input): one [104, 2*P + n_cols]
    f32 tensor; partitions 32g..32g+7 hold, for row group g:
      cols [r*P, (r+1)*P): yT of batch chunk c = 4r+g   (rounds r=0,1)
      cols [2*P, 2*P+n_cols): V replica

    Columns [0, c32) use true fp32 matmuls; columns [c32, n_cols) use
    float32r (full-rate single-pass) — the caller guarantees every value
    there is tiny enough that fp32r rounding is far below the fp32 noise
    floor of the early columns.  Walrus requires fp32r matmul operands to
    be produced as fp32r, so those live in a separate host-pre-rounded
    input tensor declared float32r end to end.
    """
    import concourse.bass as bass
    import concourse.mybir as mybir

    assert n_cols <= MM_CHUNK, "raw kernel assumes single-column-chunk output"
    f32 = mybir.dt.float32
    f32r = mybir.dt.float32r
    v_off = N_ROUNDS * P
    in_cols = v_off + c32              # f32 input: yT rounds + V[:, :c32]
    nq_cols = n_cols - c32
    inq_cols = (v_off + nq_cols) if nq_cols else 0

    nc = bass.Bass("TRN2", target_bir_lowering=False, debug=False,
                   num_devices=N_CORES)
    inp = nc.dram_tensor("inp", [3 * 32 + AR, in_cols], f32,
                         kind="ExternalInput").ap()
    inpq = None
    if nq_cols:
        inpq = nc.dram_tensor("inpq", [3 * 32 + AR, inq_cols], f32r,
                              kind="ExternalInput").ap()
    out = nc.dram_tensor("out", [ROWS, n_cols], f32,
                         kind="ExternalOutput").ap()

    with (
        nc.sbuf_tensor([3 * 32 + AR, in_cols], f32) as inp_t,
        nc.sbuf_tensor([3 * 32 + AR, max(inq_cols, 1)], f32r) as inpq_t,
        nc.sbuf_tensor([P, N_CHUNKS * n_cols], f32) as out_t,
        nc.sbuf_tensor([1, 2], f32) as scratch_t,
        nc.psum_tensor([P, N_CHUNKS, MM_CHUNK], f32) as psum_t,
        nc.semaphore() as in_sem,
        nc.semaphore() as mm_sem,
        nc.semaphore() as cpv_sem,
        nc.semaphore() as cps_sem,
        nc.semaphore() as do_sem,
        nc.semaphore() as dummy_sem,
        nc.Block() as block,
    ):
        # input split across the two HWDGE rings (sync + scalar) so the
        # HBM reads overlap; output DMAs likewise alternate rings.
        @block.sync
        def _(sync):
            if nq_cols:
                sync.dma_start(out=inpq_t[:, :inq_cols],
                               in_=inpq).then_inc(in_sem, 16)
            else:
                sync.dma_start(out=inp_t[:, v_off:],
                               in_=inp[:, v_off:]).then_inc(in_sem, 16)
            for i in range(N_CHUNKS // 2):
                c = 2 * i
                sync.wait_ge(cpv_sem, i + 1)
                sync.dma_start(
                    out=out[c * P:(c + 1) * P, :],
                    in_=out_t[:, c * n_cols:(c + 1) * n_cols],
                ).then_inc(do_sem, 16)
            sync.wait_ge(do_sem, N_CHUNKS * 16)

        @block.tensor
        def _(tensor):
            tensor.wait_ge(in_sem, 32)
            for r in range(N_ROUNDS):
                for g in range(N_GRP):
                    c = N_GRP * r + g
                    p0 = 32 * g
                    mm = tensor.matmul(
                        psum_t[:, c, :c32],
                        inp_t[p0:p0 + AR, r * P:(r + 1) * P],
                        inp_t[p0:p0 + AR, v_off:v_off + c32],
                        start=True, stop=True,
                        tile_position=(p0, 0),
                    )
                    if nq_cols:
                        # PE matmuls complete in pc order, so the inc on the
                        # second matmul covers both
                        mm = tensor.matmul(
                            psum_t[:, c, c32:n_cols],
                            inpq_t[p0:p0 + AR, r * P:(r + 1) * P],
                            inpq_t[p0:p0 + AR, v_off:v_off + nq_cols],
                            start=True, stop=True,
                            tile_position=(p0, 0),
                        )
                    mm.then_inc(mm_sem, 1)

        @block.gpsimd
        def _(gpsimd):
            gpsimd.memset(scratch_t[:, 0:1], 0.0).then_inc(dummy_sem, 1)

        @block.vector
        def _(vector):
            for i in range(N_CHUNKS // 2):
                c = 2 * i
                vector.wait_ge(mm_sem, c + 1)
                vector.tensor_copy(
                    out_t[:, c * n_cols:(c + 1) * n_cols],
                    psum_t[:, c, :n_cols],
                ).then_inc(cpv_sem, 1)

        @block.scalar
        def _(scalar):
            if nq_cols:
                scalar.dma_start(out=inp_t[:, :in_cols],
                                 in_=inp).then_inc(in_sem, 16)
            else:
                scalar.dma_start(out=inp_t[:, :v_off],
                                 in_=inp[:, :v_off]).then_inc(in_sem, 16)
            # dummy op: pull ACT_TABLE_LOAD into the input-DMA wait window
            scalar.wait_ge(dummy_sem, 1)
            scalar.copy(scratch_t[:, 1:2], scratch_t[:, 0:1])
            for i in range(N_CHUNKS // 2):
                c = 2 * i + 1
                scalar.wait_ge(mm_sem, c + 1)
                scalar.copy(
                    out_t[:, c * n_cols:(c + 1) * n_cols],
                    psum_t[:, c, :n_cols],
                ).then_inc(cps_sem, 1)
                # same-engine pipelining: make sure the copy has drained
                # before the DMA reads out_t
                scalar.wait_ge(cps_sem, i + 1)
                scalar.dma_start(
                    out=out[c * P:(c + 1) * P, :],
                    in_=out_t[:, c * n_cols:(c + 1) * n_cols],
                ).then_inc(do_sem, 16)

    return nc


def _build_nc_tile(n_cols):
    """Tile-framework fallback (any n_cols)."""
    import concourse.mybir as mybir
    import concourse.tile as tile
    from concourse import bacc

    f32 = mybir.dt.float32
    nc = bacc.Bacc("TRN2", target_bir_lowering=False, debug=False,
                   num_devices=N_CORES)
    yT = nc.dram_tensor("yT", [AR, ROWS], f32, kind="ExternalInput").ap()
    V = nc.dram_tensor("V", [AR, n_cols], f32, kind="ExternalInput").ap()
    out = nc.dram_tensor("out", [ROWS, n_cols], f32,
                         kind="ExternalOutput").ap()

    chunks = [(c, min(MM_CHUNK, n_cols - c)) for c in range(0, n_cols, MM_CHUNK)]

    with tile.TileContext(nc) as tc:
        with (
            tc.tile_pool(name="const", bufs=1) as cpool,
            tc.tile_pool(name="outs", bufs=3) as opool,
            tc.tile_pool(name="psum", bufs=8, space="PSUM") as ppool,
        ):
            yT_t = cpool.tile([AR, ROWS], f32)
            nc.sync.dma_start(yT_t[:], yT)
            V_t = cpool.tile([AR, n_cols], f32)
            nc.sync.dma_start(V_t[:], V)
            for rc in range(ROWS // P):
                ot = opool.tile([P, n_cols], f32, tag="ot")
                for c, wd in chunks:
                    ps = ppool.tile([P, MM_CHUNK], f32, tag="ps")
                    nc.tensor.matmul(
                        ps[:, :wd],
                        yT_t[:, rc * P:(rc + 1) * P],
                        V_t[:, c:c + wd],
                        start=True, stop=True,
                    )
                    nc.vector.tensor_copy(ot[:, c:c + wd], ps[:, :wd])
                nc.sync.dma_start(out[rc * P:(rc + 1) * P, :], ot[:])
    nc.compile()
    return nc


def _v_table(W):
    """V[:, t] = M^t w in float64, cast to float32.  v_{t+1}[0] = w0*v[7],
    v_{t+1}[i] = v[i-1] + w_i*v[7]."""
    w = np.asarray(W, dtype=np.float64)[0, :AR]
    V = np.zeros((AR, SEQ), dtype=np.float64)
    v = w.copy()
    for t in range(SEQ):
        V[:, t] = v
        nv = np.empty(AR)
        nv[0] = 0.0
        nv[1:] = v[:-1]
        nv += w * v[AR - 1]
        v = nv
        if not np.isfinite(v).all():
            # unstable recurrence: remaining columns pinned at f32-max scale
            V[:, t + 1:] = np.nan_to_num(v, posinf=np.finfo(np.float32).max,
                                         neginf=np.finfo(np.float32).min)[:, None]
            break
    return V.astype(np.float32)


def _round_f32r(a):
    """Pre-round to the PE's fp32r (tf32-like) input precision by dropping
    low mantissa bits.  Only used for values < 1e-10 of the output scale, so
    any reasonable guess at the exact hardware format is far below the
    comparison threshold."""
    b = a.copy().view(np.uint32)
    b &= np.uint32(0xFFFFE000)
    return b.view(np.float32)


def _pack_input(y_shard, V_full, c32):
    """Build the f32 ([104, 2*P + c32]: yT rounds + V[:, :c32]) and f32r
    ([104, 2*P + (n_cols-c32)]: rounded yT + V[:, c32:]) inputs for
    _build_nc_raw (see its docstring)."""
    n_cols = V_full.shape[1]
    v_off = N_ROUNDS * P
    yt = np.zeros((3 * 32 + AR, v_off), dtype=np.float32)
    for g in range(N_GRP):
        for r in range(N_ROUNDS):
            c = N_GRP * r + g
            yt[32 * g:32 * g + AR, r * P:(r + 1) * P] = \
                y_shard[c * P:(c + 1) * P, :].T
    vrep = np.zeros((3 * 32 + AR, n_cols), dtype=np.float32)
    for g in range(N_GRP):
        vrep[32 * g:32 * g + AR, :] = V_full
    inp = np.ascontiguousarray(
        np.concatenate([yt, vrep[:, :c32]], axis=1))
    if c32 < n_cols:
        inpq = _round_f32r(np.ascontiguousarray(
            np.concatenate([yt, vrep[:, c32:]], axis=1)))
    else:
        inpq = None
    return inp, inpq


def kernel(y, u, W):
    global LAST_RESULTS
    from concourse.bass_utils import run_bass_kernel_spmd

    y = np.ascontiguousarray(np.asarray(y, dtype=np.float32))
    Vf = _v_table(W)

    colmax = np.abs(Vf).max(axis=0)
    # columns with colmax < 1e-40 contribute at most ~1e-39 absolute (vs an
    # O(1) output scale) and the f32 reference is exactly 0 there — skip them
    nz = np.nonzero(colmax >= 1e-40)[0]
    t_last = int(nz[-1]) + 1 if len(nz) else 0
    n_cols = min(OUT_COLS, (AR + t_last + 9 + 7) & ~7)
    # fp32 -> fp32r switchover: where values fall below 1e-10 of scale
    prec = np.nonzero(colmax >= 1e-10)[0]
    t_prec = int(prec[-1]) + 1 if len(prec) else 0
    c32 = min(n_cols, (AR + t_prec + 7) & ~7)
    if n_cols - c32 < 256:
        # float32r only runs full-rate with free dim >= 256
        c32 = n_cols if n_cols < c32 + 256 and n_cols == OUT_COLS else c32
        n_cols = min(OUT_COLS, max(n_cols, c32 + 256))
        if n_cols - c32 < 256:
            c32 = n_cols

    V_full = np.zeros((AR, n_cols), dtype=np.float32)
    V_full[:, :AR] = np.eye(AR, dtype=np.float32)
    V_full[:, AR:] = Vf[:, :n_cols - AR]

    impl = os.environ.get("KERNEL_IMPL", "raw")
    if impl == "raw" and n_cols > MM_CHUNK:
        impl = "tile"                               # raw path is prefix-only

    key = (impl, n_cols, c32)
    if key not in _nc_cache:
        _nc_cache[key] = (_build_nc_raw(n_cols, c32) if impl == "raw"
                          else _build_nc_tile(n_cols))
    nc = _nc_cache[key]

    if impl == "raw":
        in_maps = []
        for i in range(N_CORES):
            inp, inpq = _pack_input(y[i * ROWS:(i + 1) * ROWS], V_full, c32)
            m = {"inp": inp}
            if inpq is not None:
                m["inpq"] = inpq
            in_maps.append(m)
    else:
        in_maps = [
            {"yT": np.ascontiguousarray(y[i * ROWS:(i + 1) * ROWS].T),
             "V": V_full}
            for i in range(N_CORES)
        ]
    LAST_RESULTS = run_bass_kernel_spmd(nc, in_maps, list(range(N_CORES)))

    out = np.zeros((BATCH, OUT_COLS), dtype=np.float32)
    for i in range(N_CORES):
        out[i * ROWS:(i + 1) * ROWS, :n_cols] = LAST_RESULTS.results[i]["out"]
    return out


# revision 22
# speedup vs baseline: 1.1556x; 1.1556x over previous
"""ARX forward kernel for Trainium2 (8 NeuronCores, data-parallel).

The reference zeroes the exogenous term, so the model is a pure linear
recurrence out[:, t] = sum_k w_k * out[:, t-8+k] with out[:, :8] = y.
Writing the 8x8 companion matrix M (carry_{t+1} = carry_t @ M) gives
pred_t = y @ (M^t w), so the whole 4096-step scan collapses into one
matmul out = y @ [I_8 | V] with V[:, t] = M^t w precomputed on host
(4096 tiny 8-vector iterations, float64).

The recurrence is stable (spectral radius ~0.77 for the 0.05-scaled
weights), so M^t w underflows float32 to exactly 0 after a few hundred
steps; both the reference scan and this kernel produce exact zeros
there.  The device therefore computes and writes only the nonzero
column prefix (determined from V at runtime) and the host pads the
remaining all-zero columns.

Sharding: pure data parallel, batch 8192 -> 1024 rows per core, W/V
replicated, per-core output gathered on host by concatenation.

Device kernel (raw bass, per core): the contraction dim is only 8, so
matmuls are packed 4x via TensorE row tiling (32x128 mode): row groups
at partitions 0/32/64/96 each hold one 128-row batch chunk's yT and a
replica of V, and 4 matmuls run concurrently in the array.  2 rounds
cover the 8 chunks; 8 PSUM banks hold the results, which DVE/ACT
copy to SBUF and HWDGE DMAs stream to DRAM.
"""

import os

import numpy as np

AR = 8
SEQ = 4096
BATCH = 8192
OUT_COLS = SEQ + AR          # 4104
N_CORES = 8
ROWS = BATCH // N_CORES      # 1024
P = 128                      # SBUF/PSUM partitions
MM_CHUNK = 512               # max fp32 matmul free dim / one PSUM bank
N_CHUNKS = ROWS // P         # 8 row chunks per core
N_GRP = 4                    # TensorE row groups (32-row tiling)
N_ROUNDS = N_CHUNKS // N_GRP

_nc_cache = {}
LAST_RESULTS = None          # BassKernelResults of the most recent run


def _build_nc_raw(n_cols, c32=None, tag=""):
    """Raw-bass program: out[1024, n_cols] = y_shard @ [I|V] (per core).

    Input layout (host-packed, see _pack_input): one [104, 2*P + n_cols]
    f32 tensor; partitions 32g..32g+7 hold, for row group g:
      cols [r*P, (r+1)*P): yT of batch chunk c = 4r+g   (rounds r=0,1)
      cols [2*P, 2*P+n_cols): V replica

    Columns [0, c32) use true fp32 matmuls; columns [c32, n_cols) use
    float32r (full-rate single-pass) — the caller guarantees every value
    there is tiny enough that fp32r rounding is far below the fp32 noise
    floor of the early columns.  Walrus requires fp32r matmul operands to
    be produced as fp32r, so those live in a separate host-pre-rounded
    input tensor declared float32r end to end.
    """
    import concourse.bass as bass
    import concourse.mybir as mybir

    assert n_cols <= MM_CHUNK, "raw kernel assumes single-column-chunk output"
    f32 = mybir.dt.float32
    f32r = mybir.dt.float32r
    v_off = N_ROUNDS * P
    in_cols = v_off + c32              # f32 input: yT rounds + V[:, :c32]
    nq_cols = n_cols - c32
    inq_cols = (v_off + nq_cols) if nq_cols else 0

    nc = bass.Bass("TRN2", target_bir_lowering=False, debug=False,
                   num_devices=N_CORES)
    inp = nc.dram_tensor("inp", [3 * 32 + AR, in_cols], f32,
                         kind="ExternalInput").ap()
    inpq = None
    if nq_cols:
        inpq = nc.dram_tensor("inpq", [3 * 32 + AR, inq_cols], f32r,
                              kind="ExternalInput").ap()
    out = nc.dram_tensor("out", [ROWS, n_cols], f32,
                         kind="ExternalOutput").ap()

    with (
        nc.sbuf_tensor([3 * 32 + AR, in_cols], f32) as inp_t,
        nc.sbuf_tensor([3 * 32 + AR, max(inq_cols, 1)], f32r) as inpq_t,
        nc.sbuf_tensor([P, N_CHUNKS * n_cols], f32) as out_t,
        nc.sbuf_tensor("scratch" + tag, [1, 2], f32) as scratch_t,
        nc.psum_tensor([P, N_CHUNKS, MM_CHUNK], f32) as psum_t,
        nc.semaphore() as in_sem,
        nc.semaphore() as mm_sem,
        nc.semaphore() as cpv_sem,
        nc.semaphore() as cps_sem,
        nc.semaphore() as do_sem,
        nc.semaphore() as dummy_sem,
        nc.Block() as block,
    ):
        # input split across the two HWDGE rings (sync + scalar) so the
        # HBM reads overlap; output DMAs likewise alternate rings.
        @block.sync
        def _(sync):
            if nq_cols:
                sync.dma_start(out=inpq_t[:, :inq_cols],
                               in_=inpq).then_inc(in_sem, 16)
            else:
                sync.dma_start(out=inp_t[:, v_off:],
                               in_=inp[:, v_off:]).then_inc(in_sem, 16)
            for c in range(6):
                # even chunks are copied by vector (cpv), odd by scalar (cps)
                sem, n = (cpv_sem, c // 2 + 1) if c % 2 == 0 else \
                    (cps_sem, c // 2 + 1)
                sync.wait_ge(sem, n)
                sync.dma_start(
                    out=out[c * P:(c + 1) * P, :],
                    in_=out_t[:, c * n_cols:(c + 1) * n_cols],
                ).then_inc(do_sem, 16)
            sync.wait_ge(do_sem, N_CHUNKS * 16)

        @block.tensor
        def _(tensor):
            tensor.wait_ge(in_sem, 32)
            for r in range(N_ROUNDS):
                for g in range(N_GRP):
                    c = N_GRP * r + g
                    p0 = 32 * g
                    mm = tensor.matmul(
                        psum_t[:, c, :c32],
                        inp_t[p0:p0 + AR, r * P:(r + 1) * P],
                        inp_t[p0:p0 + AR, v_off:v_off + c32],
                        start=True, stop=True,
                        tile_position=(p0, 0),
                    )
                    if nq_cols:
                        # PE matmuls complete in pc order, so the inc on the
                        # second matmul covers both
                        mm = tensor.matmul(
                            psum_t[:, c, c32:n_cols],
                            inpq_t[p0:p0 + AR, r * P:(r + 1) * P],
                            inpq_t[p0:p0 + AR, v_off:v_off + nq_cols],
                            start=True, stop=True,
                            tile_position=(p0, 0),
                        )
                    mm.then_inc(mm_sem, 1)

        @block.gpsimd
        def _(gpsimd):
            gpsimd.memset(scratch_t[:, 0:1], 0.0).then_inc(dummy_sem, 1)

        @block.vector
        def _(vector):
            for i in range(N_CHUNKS // 2):
                c = 2 * i
                vector.wait_ge(mm_sem, c + 1)
                vector.tensor_copy(
                    out_t[:, c * n_cols:(c + 1) * n_cols],
                    psum_t[:, c, :n_cols],
                ).then_inc(cpv_sem, 1)

        @block.scalar
        def _(scalar):
            if nq_cols:
                scalar.dma_start(out=inp_t[:, :in_cols],
                                 in_=inp).then_inc(in_sem, 16)
            else:
                scalar.dma_start(out=inp_t[:, :v_off],
                                 in_=inp[:, :v_off]).then_inc(in_sem, 16)
            # dummy op: pull ACT_TABLE_LOAD into the input-DMA wait window
            scalar.wait_ge(dummy_sem, 1)
            scalar.copy(scratch_t[:, 1:2], scratch_t[:, 0:1])
            for i in range(N_CHUNKS // 2):
                c = 2 * i + 1
                scalar.wait_ge(mm_sem, c + 1)
                scalar.copy(
                    out_t[:, c * n_cols:(c + 1) * n_cols],
                    psum_t[:, c, :n_cols],
                ).then_inc(cps_sem, 1)
            # chunks 6 and 7 stream out on the scalar HWDGE ring (sync's
            # ring carries chunks 0-5); cps_sem>=4 also proves scalar's own
            # copy pipeline (chunk 7) has drained before the DMA reads out_t
            scalar.wait_ge(cpv_sem, 4)
            scalar.dma_start(
                out=out[6 * P:7 * P, :],
                in_=out_t[:, 6 * n_cols:7 * n_cols],
            ).then_inc(do_sem, 16)
            scalar.wait_ge(cps_sem, 4)
            scalar.dma_start(
                out=out[7 * P:8 * P, :],
                in_=out_t[:, 7 * n_cols:8 * n_cols],
            ).then_inc(do_sem, 16)

    return nc


def _build_nc_tile(n_cols):
    """Tile-framework fallback (any n_cols)."""
    import concourse.mybir as mybir
    import concourse.tile as tile
    from concourse import bacc

    f32 = mybir.dt.float32
    nc = bacc.Bacc("TRN2", target_bir_lowering=False, debug=False,
                   num_devices=N_CORES)
    yT = nc.dram_tensor("yT", [AR, ROWS], f32, kind="ExternalInput").ap()
    V = nc.dram_tensor("V", [AR, n_cols], f32, kind="ExternalInput").ap()
    out = nc.dram_tensor("out", [ROWS, n_cols], f32,
                         kind="ExternalOutput").ap()

    chunks = [(c, min(MM_CHUNK, n_cols - c)) for c in range(0, n_cols, MM_CHUNK)]

    with tile.TileContext(nc) as tc:
        with (
            tc.tile_pool(name="const", bufs=1) as cpool,
            tc.tile_pool(name="outs", bufs=3) as opool,
            tc.tile_pool(name="psum", bufs=8, space="PSUM") as ppool,
        ):
            yT_t = cpool.tile([AR, ROWS], f32)
            nc.sync.dma_start(yT_t[:], yT)
            V_t = cpool.tile([AR, n_cols], f32)
            nc.sync.dma_start(V_t[:], V)
            for rc in range(ROWS // P):
                ot = opool.tile([P, n_cols], f32, tag="ot")
                for c, wd in chunks:
                    ps = ppool.tile([P, MM_CHUNK], f32, tag="ps")
                    nc.tensor.matmul(
                        ps[:, :wd],
                        yT_t[:, rc * P:(rc + 1) * P],
                        V_t[:, c:c + wd],
                        start=True, stop=True,
                    )
                    nc.vector.tensor_copy(ot[:, c:c + wd], ps[:, :wd])
                nc.sync.dma_start(out[rc * P:(rc + 1) * P, :], ot[:])
    nc.compile()
    return nc


def _v_table(W):
    """V[:, t] = M^t w in float64, cast to float32.  v_{t+1}[0] = w0*v[7],
    v_{t+1}[i] = v[i-1] + w_i*v[7]."""
    w = np.asarray(W, dtype=np.float64)[0, :AR]
    V = np.zeros((AR, SEQ), dtype=np.float64)
    v = w.copy()
    for t in range(SEQ):
        V[:, t] = v
        nv = np.empty(AR)
        nv[0] = 0.0
        nv[1:] = v[:-1]
        nv += w * v[AR - 1]
        v = nv
        if not np.isfinite(v).all():
            # unstable recurrence: remaining columns pinned at f32-max scale
            V[:, t + 1:] = np.nan_to_num(v, posinf=np.finfo(np.float32).max,
                                         neginf=np.finfo(np.float32).min)[:, None]
            break
    return V.astype(np.float32)


def _round_f32r(a):
    """Pre-round to the PE's fp32r (tf32-like) input precision by dropping
    low mantissa bits.  Only used for values < 1e-10 of the output scale, so
    any reasonable guess at the exact hardware format is far below the
    comparison threshold."""
    b = a.copy().view(np.uint32)
    b &= np.uint32(0xFFFFE000)
    return b.view(np.float32)


def _pack_input(y_shard, V_full, c32):
    """Build the f32 ([104, 2*P + c32]: yT rounds + V[:, :c32]) and f32r
    ([104, 2*P + (n_cols-c32)]: rounded yT + V[:, c32:]) inputs for
    _build_nc_raw (see its docstring)."""
    n_cols = V_full.shape[1]
    v_off = N_ROUNDS * P
    yt = np.zeros((3 * 32 + AR, v_off), dtype=np.float32)
    for g in range(N_GRP):
        for r in range(N_ROUNDS):
            c = N_GRP * r + g
            yt[32 * g:32 * g + AR, r * P:(r + 1) * P] = \
                y_shard[c * P:(c + 1) * P, :].T
    vrep = np.zeros((3 * 32 + AR, n_cols), dtype=np.float32)
    for g in range(N_GRP):
        vrep[32 * g:32 * g + AR, :] = V_full
    inp = np.ascontiguousarray(
        np.concatenate([yt, vrep[:, :c32]], axis=1))
    if c32 < n_cols:
        inpq = _round_f32r(np.ascontiguousarray(
            np.concatenate([yt, vrep[:, c32:]], axis=1)))
    else:
        inpq = None
    return inp, inpq


def kernel(y, u, W):
    global LAST_RESULTS
    from concourse.bass_utils import run_bass_kernel_spmd

    y = np.ascontiguousarray(np.asarray(y, dtype=np.float32))
    Vf = _v_table(W)

    colmax = np.abs(Vf).max(axis=0)
    # columns with colmax < 1e-40 contribute at most ~1e-39 absolute (vs an
    # O(1) output scale) and the f32 reference is exactly 0 there — skip them
    nz = np.nonzero(colmax >= 1e-40)[0]
    t_last = int(nz[-1]) + 1 if len(nz) else 0
    n_cols = min(OUT_COLS, (AR + t_last + 9 + 7) & ~7)
    # fp32 -> fp32r switchover: where values fall below 1e-10 of scale
    prec = np.nonzero(colmax >= 1e-10)[0]
    t_prec = int(prec[-1]) + 1 if len(prec) else 0
    c32 = min(n_cols, (AR + t_prec + 7) & ~7)
    if n_cols - c32 < 256:
        # float32r only runs full-rate with free dim >= 256
        c32 = n_cols if n_cols < c32 + 256 and n_cols == OUT_COLS else c32
        n_cols = min(OUT_COLS, max(n_cols, c32 + 256))
        if n_cols - c32 < 256:
            c32 = n_cols

    V_full = np.zeros((AR, n_cols), dtype=np.float32)
    V_full[:, :AR] = np.eye(AR, dtype=np.float32)
    V_full[:, AR:] = Vf[:, :n_cols - AR]

    impl = os.environ.get("KERNEL_IMPL", "raw")
    if impl == "raw" and n_cols > MM_CHUNK:
        impl = "tile"                               # raw path is prefix-only

    key = (impl, n_cols, c32)
    if key not in _nc_cache:
        _nc_cache[key] = (_build_nc_raw(n_cols, c32) if impl == "raw"
                          else _build_nc_tile(n_cols))
    nc = _nc_cache[key]

    if impl == "raw":
        in_maps = []
        for i in range(N_CORES):
            inp, inpq = _pack_input(y[i * ROWS:(i + 1) * ROWS], V_full, c32)
            m = {"inp": inp}
            if inpq is not None:
                m["inpq"] = inpq
            in_maps.append(m)
    else:
        in_maps = [
            {"yT": np.ascontiguousarray(y[i * ROWS:(i + 1) * ROWS].T),
             "V": V_full}
            for i in range(N_CORES)
        ]
    LAST_RESULTS = run_bass_kernel_spmd(nc, in_maps, list(range(N_CORES)))

    out = np.zeros((BATCH, OUT_COLS), dtype=np.float32)
    for i in range(N_CORES):
        out[i * ROWS:(i + 1) * ROWS, :n_cols] = LAST_RESULTS.results[i]["out"]
    return out
